# revision 4
# baseline (speedup 1.0000x reference)
"""Trainium2 Bass kernel for nn_CAM_Multimodal_Module (retrieval_knn).

Per batch b:
    energy[i, j] = <rgb[b, i, :], depth[b, j, :]>   (contraction over H*W)
    cl[i] = argmax_j energy[i, j]
    out[b, i, :] = rgb[b, i, :] + depth[b, cl[i], :]

Sharding: pure data parallel, 2 batches per core across 8 cores.

Energy path (fp16x3): E ~= qh.kh + qh.kl + ql.kh with fp32 PSUM
accumulation (max energy error ~3e-4 vs fp64, vs min top-2 gap 1.27e-3
-> argmax exact). Same PSUM accumulation order as the validated v1.

v3 architecture: transpose-then-split. The PE transposes the RAW fp32
data (2 cy/row — the same total PE cycles as transposing two fp16
halves at 1 cy/row), and the fp16 split happens in transposed space as
the PSUM->SBUF copy itself:
    qh^T = ACT copy (fp32 PSUM -> fp16 SBUF, rounds)
    ql^T = DVE sub  (fp32 PSUM - qh^T -> fp16)
    kh^T/kl^T likewise.
This removes the whole pre-split stage: loads feed the PE directly, so
piece pipelining has no cross-engine serialization, and SBUF pressure
drops (no split tiles).

Schedule: piece-pipelined loads (small first piece + PE pstate warm-up
-> PE starts hot at ~3.5us); per-piece batched q-then-k emission (no
engine head-of-line stalls); batch 0 chunk-major while loads stream with
a tile-major window at the end; batch 1 fully tile-major (energy[t]
completes ~11.5us apart so each argmax+gather+store overlaps remaining
matmuls). Tail gathers chunked at 3072 B (4608 B is the HW corruption
limit) with pipelined stores. Queues: loads+stores on SP, gathers on
Pool (SWDGE). TimelineSim: 140912 ns (baseline fp16x3 kernel: 167637).
"""

import numpy as np
from contextlib import ExitStack

import concourse.bass as bass
import concourse.tile as tile
from concourse import bacc, mybir
from concourse.bass_utils import run_bass_kernel_spmd
from concourse.masks import make_identity
from concourse._compat import with_exitstack

B, C, H, W = 16, 512, 48, 48
HW = H * W              # 2304
NCORES = 8
NB = B // NCORES        # 2 batches per core
P = 128
NT = C // P             # 4 channel tiles
NCH = HW // P           # 18 contraction chunks
F32 = mybir.dt.float32
F16 = mybir.dt.float16
U32 = mybir.dt.uint32

TMAJ_B0 = 8             # tile-major window (chunks) for batch 0
GATHER_PIECES = 3       # gather/store pieces per tile (768 cols = 3072 B <= 4608)
WARMUP = 8              # PE pstate warm-up transposes during load latency
MERGE_QH = 0
PW = 512                # dep piece tile width per channel tile
PIECES = [(0, 128), (128, 384), (512, 512), (1024, 512), (1536, 512), (2048, 256)]

_NC_CACHE = {}


@with_exitstack
def _body(ctx, tc, out_d, rgb_d, dep_d):
    nc = tc.nc
    consts = ctx.enter_context(tc.tile_pool(name="consts", bufs=1))
    rgbp = ctx.enter_context(tc.tile_pool(name="rgbp", bufs=1))
    depp = ctx.enter_context(tc.tile_pool(name="depp", bufs=6))
    tposep = ctx.enter_context(tc.tile_pool(name="tposep", bufs=1))
    psum_t = ctx.enter_context(tc.tile_pool(name="psum_t", bufs=2, space="PSUM"))
    psum_e = ctx.enter_context(tc.tile_pool(name="psum_e", bufs=1, space="PSUM"))
    argp = ctx.enter_context(tc.tile_pool(name="argp", bufs=4))

    ident32 = consts.tile([P, P], F32, tag="ident32")
    make_identity(nc, ident32[:])
    warm = psum_e.tile([P, C], F32, tag="energy0", name="warm")
    for _ in range(WARMUP):
        # PE pstate warm-up during the load latency; energy bank 0 is
        # reset by its first real matmul (start=True) afterwards.
        nc.tensor.transpose(warm[:, 0:P], ident32[:], ident32[:])

    # all-chunk transposed fp16 tiles (shared across batches; deps serialize)
    qT = tposep.tile([P, NCH * C], F16, tag="qT")       # qh^T chunk-blocks of 512
    qlT = tposep.tile([P, NCH * C], F16, tag="qlT")     # ql^T chunk-blocks of 512
    kT = tposep.tile([P, NCH * 2 * C], F16, tag="kT")   # [kh^T | kl^T] blocks of 1024

    rgb_sb = {}

    def emit_load_transpose_split(b, energy=None, mm_upto=0):
        rgb = rgbp.tile([P, NT * HW], F32, tag=f"rgb{b}")
        rgb_sb[b] = rgb
        rgb3 = rgb[:].rearrange("p (t c) -> p t c", t=NT)
        rgb_dram = rgb_d[b * C : (b + 1) * C, :].rearrange("(t p) c -> p t c", p=P)
        dep_dram = dep_d[b * C : (b + 1) * C, :].rearrange("(t p) c -> p t c", p=P)
        deps = []
        for pi, (c0, w) in enumerate(PIECES):
            cs = slice(c0, c0 + w)
            nc.sync.dma_start(rgb3[:, :, cs], rgb_dram[:, :, cs])
            dep = depp.tile([P, NT * PW], F32, tag="dep", name=f"dep_b{b}p{pi}")
            dep3 = dep[:].rearrange("p (t c) -> p t c", t=NT)[:, :, 0:w]
            nc.sync.dma_start(dep3, dep_dram[:, :, cs])
            deps.append(dep)
        prev_chunks = []
        for pi, (c0, w) in enumerate(PIECES):
            dep = deps[pi]
            chunks = list(range(c0 // P, (c0 + w) // P))
            # q side for the whole piece first (never blocked by the dep
            # arrival skew), then k side, then fill matmuls of the previous
            # piece -- keeps every engine queue free of head-of-line stalls.
            for ch in chunks:
                ps_q = psum_t.tile([P, C], F32, tag="ps_q", name=f"ps_q_b{b}c{ch}")
                for t in range(NT):
                    nc.tensor.transpose(
                        ps_q[:, t * P : (t + 1) * P],
                        rgb[:, t * HW + ch * P : t * HW + (ch + 1) * P],
                        ident32[:],
                    )
                qh_s = qT[:, ch * C : (ch + 1) * C]
                nc.scalar.copy(qh_s, ps_q[:])
                nc.vector.tensor_sub(qlT[:, ch * C : (ch + 1) * C], ps_q[:], qh_s)
            for ch in chunks:
                lc = ch * P - c0
                ps_k = psum_t.tile([P, C], F32, tag="ps_k", name=f"ps_k_b{b}c{ch}")
                for t in range(NT):
                    nc.tensor.transpose(
                        ps_k[:, t * P : (t + 1) * P],
                        dep[:, t * PW + lc : t * PW + lc + P],
                        ident32[:],
                    )
                kh_s = kT[:, ch * 2 * C : ch * 2 * C + C]
                nc.scalar.copy(kh_s, ps_k[:])
                nc.vector.tensor_sub(
                    kT[:, ch * 2 * C + C : (ch + 1) * 2 * C], ps_k[:], kh_s
                )
            if energy is not None:
                for ch in prev_chunks:
                    if ch < mm_upto:
                        for t in range(NT):
                            emit_matmul(t, ch, energy)
            prev_chunks = chunks

    def emit_matmul(t, ch, energy):
        qhT_t = qT[:, ch * C + t * P : ch * C + (t + 1) * P]
        qlT_t = qlT[:, ch * C + t * P : ch * C + (t + 1) * P]
        khT = kT[:, ch * 2 * C : ch * 2 * C + C]
        klT = kT[:, ch * 2 * C + C : (ch + 1) * 2 * C]
        if MERGE_QH:
            # one 1024-row pass over [khT|klT]; the repeated out AP
            # accumulates rows 512-1023 onto the same PSUM columns
            e = energy[t][:]
            e2 = bass.AP(e.tensor, e.offset, [list(p) for p in e.ap][:1] + [[0, 2]] + [list(p) for p in e.ap][1:])
            nc.tensor.matmul(e2, lhsT=qhT_t, rhs=kT[:, ch * 2 * C : (ch + 1) * 2 * C],
                             start=(ch == 0), stop=False, skip_group_check=True)
        else:
            nc.tensor.matmul(energy[t][:], lhsT=qhT_t, rhs=khT,
                             start=(ch == 0), stop=False)
            nc.tensor.matmul(energy[t][:], lhsT=qhT_t, rhs=klT,
                             start=False, stop=False)
        nc.tensor.matmul(energy[t][:], lhsT=qlT_t, rhs=khT,
                         start=False, stop=(ch == NCH - 1))

    def make_energy(b):
        return [
            psum_e.tile([P, C], F32, tag=f"energy{t}", name=f"energy_b{b}t{t}")
            for t in range(NT)
        ]

    def emit_matmuls(b, energy, tmaj, ch_start=0):
        for ch in range(ch_start, NCH - tmaj):
            for t in range(NT):
                emit_matmul(t, ch, energy)
        for t in range(NT):
            for ch in range(NCH - tmaj, NCH):
                emit_matmul(t, ch, energy)
        return energy

    def emit_tail(b, energy, last_batch):
        rgb = rgb_sb[b]
        for t in range(NT):
            mx8 = argp.tile([P, 8], F32, tag="mx8", name=f"mx8_b{b}t{t}")
            nc.vector.max(mx8[:], energy[t][:])
            idx8 = argp.tile([P, 8], U32, tag="idx8", name=f"idx8_b{b}t{t}")
            nc.vector.max_index(idx8[:], mx8[:], energy[t][:])
            npieces = GATHER_PIECES
            pw = HW // npieces
            for gp in range(npieces):
                c0 = gp * pw
                nc.gpsimd.indirect_dma_start(
                    out=rgb[:, t * HW + c0 : t * HW + c0 + pw],
                    out_offset=None,
                    in_=dep_d[:],
                    in_offset=bass.IndirectOffsetOnAxis(ap=idx8[:, 0:1], axis=0),
                    element_offset=b * C * HW + c0,
                    compute_op=mybir.AluOpType.add,
                )
                nc.sync.dma_start(
                    out_d[b * C + t * P : b * C + (t + 1) * P, c0 : c0 + pw],
                    rgb[:, t * HW + c0 : t * HW + c0 + pw],
                )

    # phase order: b0's chunk-major matmuls interleave into its transpose
    # stream (PE never starves on the load ramp); b1 loads/transposes/splits
    # before b0's tail; b0's tail overlaps b1's tile-major matmul phase.
    MM_UPTO = NCH - TMAJ_B0
    energy0 = make_energy(0)
    emit_load_transpose_split(0, energy=energy0, mm_upto=MM_UPTO)
    emit_matmuls(0, energy0, TMAJ_B0, ch_start=MM_UPTO)
    emit_load_transpose_split(1)
    emit_tail(0, energy0, last_batch=False)
    energy1 = make_energy(1)
    emit_matmuls(1, energy1, NCH)
    emit_tail(1, energy1, last_batch=True)


def _build():
    nc = bacc.Bacc("TRN2", target_bir_lowering=False, debug=False)
    rgb_d = nc.dram_tensor("rgb", [NB * C, HW], F32, kind="ExternalInput")
    dep_d = nc.dram_tensor("depth", [NB * C, HW], F32, kind="ExternalInput")
    out_d = nc.dram_tensor("out", [NB * C, HW], F32, kind="ExternalOutput")
    with tile.TileContext(nc) as tc:
        _body(tc, out_d.ap(), rgb_d.ap(), dep_d.ap())
    nc.compile()
    return nc


def get_nc():
    if "nc" not in _NC_CACHE:
        _NC_CACHE["nc"] = _build()
    return _NC_CACHE["nc"]


def make_in_maps(rgb, depth):
    rgb = np.ascontiguousarray(np.asarray(rgb, dtype=np.float32)).reshape(B, C, HW)
    depth = np.ascontiguousarray(np.asarray(depth, dtype=np.float32)).reshape(B, C, HW)
    in_maps = []
    for i in range(NCORES):
        sl = slice(i * NB, (i + 1) * NB)
        in_maps.append(
            {
                "rgb": np.ascontiguousarray(rgb[sl]).reshape(NB * C, HW),
                "depth": np.ascontiguousarray(depth[sl]).reshape(NB * C, HW),
            }
        )
    return in_maps


def kernel(rgb, depth):
    nc = get_nc()
    in_maps = make_in_maps(rgb, depth)
    res = run_bass_kernel_spmd(nc, in_maps, core_ids=list(range(NCORES)))
    outs = [res.results[i]["out"].reshape(NB, C, H, W) for i in range(NCORES)]
    return np.concatenate(outs, axis=0)
